# revision 35
# baseline (speedup 1.0000x reference)
"""Trainium2 Bass kernel for Bahdanau-style attention (nn_Attention).

Computation (per batch b):
  attn1 = enc_out @ W_enc + b_enc          # [HW, ATTN]
  attn2 = dec_h @ W_dec + b_dec            # [ATTN]
  score = relu(attn1 + attn2)              # [HW, ATTN]
  logits = score @ W_v (+ b_v)             # [HW]  (b_v dropped: softmax-invariant)
  alpha = softmax(logits)                  # [HW]
  context = alpha @ enc_out                # [ENC]
Returns (context [B, ENC] f32, alpha [B, HW] f32).

Strategy: pure data-parallel over batch across 8 NeuronCores (64 batches/core),
batches processed in pipelined groups of 4:
  - enc tiles cast-DMA'd HBM f32 -> SBUF bf16 natural layout (SWDGE cast),
    as contiguous group-row tiles (6x128 + 1x16 rows per group).
  - encT (the moving operand of the big matmul) built by PE transposes written
    as REGULAR identity matmuls (exact, and unlike is_transpose they count as
    PE activity for the HAM clock gate); PSUM f32 evacuated with an fp8 cast
    split across DVE/ACT.
  - attn1^T = W_enc.T @ enc.T in fp8e4m3 with DoubleRow perf mode (2 e-chunks
    contracted per pass via the 3D-AP pair form); ~2x PE throughput vs bf16 at
    ~1.3e-2 final rel err (vs 2.5e-3 all-bf16).
  - bias (b_enc + b_dec + attn2_b) folded into the PSUM evacuation on ACT
    (per-partition bias + relu + cast to bf16 score).
  - logits via W_v-stationary matmuls; each batch's logits land on PSUM
    partition 32*j via tile_position, so softmax runs batched on one tile
    (reduce_max(negate) -> Exp with bias and fused accum_out -> reciprocal).
  - alpha transposed back to columns by one PE matmul per row-half; zero-padded
    block-diagonal A tiles kill cross-batch terms so the context accumulates a
    whole group in one PSUM group per 512-col chunk.
  - Issue order pipelines groups: loads lead by 2 groups; the LDW-heavy
    transpose packets for group g+1 are interleaved with the dense context
    matmuls of group g-1 to keep the HAM busy-fraction up; attn1 stays a dense
    fp8 block.
"""

import sys

if "/opt/trn_rl_repo" not in sys.path:
    sys.path.insert(0, "/opt/trn_rl_repo")

import numpy as np

import concourse.bass as bass
import concourse.tile as tile
from concourse import bacc, mybir
from concourse.bass_utils import run_bass_kernel_spmd
from concourse.masks import make_identity

N_CORES = 8
B, HW, ENC, DEC, ATTN = 512, 196, 2048, 512, 512
BL = B // N_CORES  # 64 batches per core
G = 4              # batches per group
NG = BL // G       # 16 groups
HW0 = 128
HW1 = HW - HW0     # 68
OUTW = ENC + HW    # context (2048) + alpha (196)

FP32 = mybir.dt.float32
BF16 = mybir.dt.bfloat16
FP8 = mybir.dt.float8e4
DR = mybir.MatmulPerfMode.DoubleRow
AX = mybir.AxisListType.X
AF = mybir.ActivationFunctionType

_CACHE = {}


def build():
    from contextlib import ExitStack

    nc = bacc.Bacc(
        "TRN2", target_bir_lowering=False, debug=False, num_devices=N_CORES
    )
    enc_d = nc.declare_dram_parameter("enc_out", [BL, HW, ENC], FP32, isOutput=False)
    dec_d = nc.declare_dram_parameter("dec_h", [BL, DEC], FP32, isOutput=False)
    wenc_d = nc.declare_dram_parameter("W_enc", [ENC, ATTN], FP32, isOutput=False)
    benc_d = nc.declare_dram_parameter("b_enc", [ATTN], FP32, isOutput=False)
    wdec_d = nc.declare_dram_parameter("W_dec", [DEC, ATTN], FP32, isOutput=False)
    bdec_d = nc.declare_dram_parameter("b_dec", [ATTN], FP32, isOutput=False)
    wv_d = nc.declare_dram_parameter("W_v", [ATTN], FP32, isOutput=False)
    out_d = nc.declare_dram_parameter("out", [BL, OUTW], FP32, isOutput=True)

    with tile.TileContext(nc) as tc:
        with ExitStack() as ctx:
            singles = ctx.enter_context(tc.tile_pool(name="singles", bufs=1))

            ident_bf = singles.tile([128, 128], BF16)
            make_identity(nc, ident_bf)

            # per-ATTN-chunk column layouts of the small vectors
            benc_t = singles.tile([128, 4], FP32)
            nc.gpsimd.dma_start(
                out=benc_t, in_=benc_d.rearrange("(ac p) -> p ac", p=128)
            )
            bdec_t = singles.tile([128, 4], FP32)
            nc.gpsimd.dma_start(
                out=bdec_t, in_=bdec_d.rearrange("(ac p) -> p ac", p=128)
            )
            bias_vec = singles.tile([128, 4], FP32)
            nc.vector.tensor_add(bias_vec, benc_t, bdec_t)

            wv_f = singles.tile([128, 4], FP32)
            nc.gpsimd.dma_start(out=wv_f, in_=wv_d.rearrange("(ac p) -> p ac", p=128))
            wv_bf = singles.tile([128, 4], BF16)
            nc.vector.tensor_copy(wv_bf, wv_f)

            # ---- attn2 / bias precompute: biasT[:, ac, b] = (dec_h @ W_dec + b_dec + b_enc)^T
            biasT = singles.tile([128, 4, BL], FP32)
            with (
                tc.tile_pool(name="pre", bufs=1) as pre,
                tc.tile_pool(name="pre_ps", bufs=2, space="PSUM") as pre_ps,
            ):
                dec_sb = pre.tile([BL, DEC], BF16)
                nc.gpsimd.dma_start(out=dec_sb, in_=dec_d[:, :])
                wdec_sb = pre.tile([128, 4, ATTN], BF16)
                for dc in range(4):
                    nc.gpsimd.dma_start(
                        out=wdec_sb[:, dc, :], in_=wdec_d[dc * 128 : (dc + 1) * 128, :]
                    )
                dechT = pre.tile([128, 4, BL], BF16)
                for dc in range(4):
                    ps = pre_ps.tile([128, BL], FP32, tag="prepst")
                    nc.tensor.matmul(
                        ps,
                        dec_sb[:, dc * 128 : (dc + 1) * 128],
                        ident_bf[0:BL, 0:BL],
                        start=True,
                        stop=True,
                    )
                    nc.vector.tensor_copy(dechT[:, dc, :], ps)
                for ac in range(4):
                    ps2 = pre_ps.tile([128, BL], FP32, tag="preps")
                    for dc in range(4):
                        nc.tensor.matmul(
                            ps2,
                            wdec_sb[:, dc, ac * 128 : (ac + 1) * 128],
                            dechT[:, dc, :],
                            start=(dc == 0),
                            stop=(dc == 3),
                        )
                    nc.vector.tensor_scalar_add(
                        biasT[:, ac, :], ps2, bias_vec[:, ac : ac + 1]
                    )

            # ---- main pools
            nat_p = ctx.enter_context(tc.tile_pool(name="nat", bufs=18))
            natS_p = ctx.enter_context(tc.tile_pool(name="natS", bufs=3))
            enct_p = ctx.enter_context(tc.tile_pool(name="enct", bufs=2))
            score_p = ctx.enter_context(tc.tile_pool(name="score", bufs=2))
            smax_p = ctx.enter_context(tc.tile_pool(name="smax", bufs=2))
            outs_p = ctx.enter_context(tc.tile_pool(name="outs", bufs=2))
            ps_a = ctx.enter_context(tc.tile_pool(name="ps_a", bufs=3, space="PSUM"))
            ps_t = ctx.enter_context(tc.tile_pool(name="ps_t", bufs=2, space="PSUM"))
            ps_at_p = ctx.enter_context(tc.tile_pool(name="ps_at", bufs=1, space="PSUM"))
            ps_c_p = ctx.enter_context(tc.tile_pool(name="ps_c", bufs=2, space="PSUM"))

            nat_tiles = {}
            enct_tiles = {}
            score_tiles = {}
            a_tiles = {}
            alpha_tiles = {}

            GR = G * HW          # 784 rows per group
            NT_FULL = GR // 128  # 6 full tiles
            TAIL = GR - NT_FULL * 128  # 16
            TSIZES = [128] * NT_FULL + [TAIL]
            encF = enc_d.rearrange("b s e -> (b s) e")

            def issue_loads(g):
                tiles = []
                for t, p in enumerate(TSIZES):
                    pool = nat_p if p == 128 else natS_p
                    nt = pool.tile([p, ENC], BF16, tag=f"nat{'S' if p < 128 else '0'}")
                    r0 = g * GR + t * 128
                    nc.gpsimd.dma_start(out=nt, in_=encF[r0 : r0 + p, :])
                    tiles.append(nt)
                nat_tiles[g] = tiles

            def transpose_packets(g):
                """Yield closures: 4 transpose matmuls + 1 evac each."""
                enct = enct_p.tile([128, 16, G * HW], FP8)
                enct_tiles[g] = enct

                def pkt(t, kq):
                    nt = nat_tiles[g][t]
                    p = TSIZES[t]
                    ps0 = ps_t.tile([128, 4 * p], FP32, tag="pst")
                    for u in range(4):
                        kc = kq * 4 + u
                        nc.tensor.matmul(
                            ps0[:, u * p : (u + 1) * p],
                            nt[:, kc * 128 : (kc + 1) * 128],
                            ident_bf[0:p, 0:p],
                            start=True,
                            stop=True,
                        )
                    ev = nc.scalar.copy if t in (1, 4) else nc.vector.tensor_copy
                    ev(
                        enct[:, kq * 4 : (kq + 1) * 4, t * 128 : t * 128 + p],
                        ps0.rearrange("p (u c) -> p u c", u=4),
                    )

                for t in range(len(TSIZES)):
                    for kq in range(4):
                        yield lambda t=t, kq=kq: pkt(t, kq)

            def issue_transpose(g):
                for pkt in transpose_packets(g):
                    pkt()

            def issue_attn1(g):
                enct = enct_tiles[g]
                sco = score_p.tile([128, 4, G * HW], BF16, tag="score")
                score_tiles[g] = sco
                half_n = G * HW // 2  # 392 = 2 batches
                for ac in range(4):
                    for half in range(2):
                        ps = ps_a.tile([128, half_n], FP32, tag="psa")
                        for kc2 in range(8):
                            nc.tensor.matmul(
                                ps,
                                w8[:, 2 * kc2 : 2 * kc2 + 2, ac * 128 : (ac + 1) * 128],
                                enct[:, 2 * kc2 : 2 * kc2 + 2, half * half_n : (half + 1) * half_n],
                                start=(kc2 == 0),
                                stop=(kc2 == 7),
                                perf_mode=DR,
                            )
                        for j2 in range(2):
                            j = half * 2 + j2
                            b = g * G + j
                            nc.scalar.activation(
                                out=sco[:, ac, j * HW : (j + 1) * HW],
                                in_=ps[:, j2 * HW : (j2 + 1) * HW],
                                func=AF.Relu,
                                bias=biasT[:, ac, b : b + 1],
                            )

            def issue_logits_softmax(g):
                sco = score_tiles[g]
                ps_lg = ps_c_p.tile([97, HW], FP32, tag="psc")
                for j in range(G):
                    for ac in range(4):
                        nc.tensor.matmul(
                            ps_lg[32 * j : 32 * j + 1, :],
                            wv_bf[:, ac : ac + 1],
                            sco[:, ac, j * HW : (j + 1) * HW],
                            start=(ac == 0),
                            stop=(ac == 3),
                            tile_position=(0, 32 * j),
                        )
                st = smax_p.tile([97, 4], FP32, tag="smx")
                ex = smax_p.tile([97, HW], FP32, tag="ex")
                alpha_full = smax_p.tile([97, HW], FP32, tag="alpha")
                alpha_tiles[g] = alpha_full
                nc.vector.reduce_max(st[:, 0:1], ps_lg, axis=AX, negate=True)
                nc.scalar.activation(
                    out=ex,
                    in_=ps_lg,
                    func=AF.Exp,
                    bias=st[:, 0:1],
                    accum_out=st[:, 1:2],
                )
                nc.vector.reciprocal(st[:, 2:3], st[:, 1:2])
                nc.vector.tensor_scalar_mul(alpha_full, ex, st[:, 2:3])
                for j in range(G):
                    bg = g * G + j
                    nc.sync.dma_start(
                        out=out_d[bg : bg + 1, ENC : ENC + HW],
                        in_=alpha_full[32 * j : 32 * j + 1, :],
                    )

            def issue_alpha_t(g):
                alpha_full = alpha_tiles[g]
                # diagonal alpha at partitions {0,32,64,96}: af[32j, r] =
                # alpha_j(r - j*HW) within batch j's row range, else 0
                af = smax_p.tile([97, GR], BF16, tag="aflat")
                nc.vector.memset(af, 0.0)
                for j in range(G):
                    nc.vector.tensor_copy(
                        af[32 * j : 32 * j + 1, j * HW : (j + 1) * HW],
                        alpha_full[32 * j : 32 * j + 1, :],
                    )
                a_sb = smax_p.tile([128, len(TSIZES), G], BF16, tag="asb")
                a_tiles[g] = a_sb
                for t, p in enumerate(TSIZES):
                    ps_at = ps_at_p.tile([128, 98], FP32, tag="psat")
                    nc.tensor.matmul(
                        ps_at[0:p, 0:97],
                        af[:, t * 128 : t * 128 + p],
                        ident_bf[0:97, 0:97],
                        start=True,
                        stop=True,
                    )
                    nc.vector.tensor_copy(a_sb[0:p, t, :], ps_at[0:p, 0:97:32])

            def context_packets(g):
                a_sb = a_tiles[g]
                ctx_sb = outs_p.tile([G, ENC], FP32, tag="ctx")

                def nchunk(nch):
                    ps_c = ps_c_p.tile([G, 512], FP32, tag="psc")
                    for t, p in enumerate(TSIZES):
                        nc.tensor.matmul(
                            ps_c,
                            a_sb[0:p, t, :],
                            nat_tiles[g][t][:, nch * 512 : (nch + 1) * 512],
                            start=(t == 0),
                            stop=(t == len(TSIZES) - 1),
                        )
                    nc.scalar.copy(ctx_sb[:, nch * 512 : (nch + 1) * 512], ps_c)

                for nch in range(4):
                    yield lambda nch=nch: nchunk(nch)
                yield lambda: nc.sync.dma_start(
                    out=out_d[g * G : (g + 1) * G, 0:ENC], in_=ctx_sb
                )

            def issue_context(g):
                for pkt in context_packets(g):
                    pkt()

            issue_loads(0)
            # W_enc chunks: bf16 cast-DMA load, then fp8 copy for DoubleRow matmuls
            w_bf = singles.tile([128, 16, ATTN], BF16)
            w8 = singles.tile([128, 16, ATTN], FP8)
            for kc in range(16):
                nc.gpsimd.dma_start(
                    out=w_bf[:, kc, :], in_=wenc_d[kc * 128 : (kc + 1) * 128, :]
                )
                nc.vector.tensor_copy(w8[:, kc, :], w_bf[:, kc, :])

            issue_loads(1)
            issue_transpose(0)
            for g in range(NG):
                if g + 2 < NG:
                    issue_loads(g + 2)
                issue_attn1(g)
                if g >= 1:
                    issue_alpha_t(g - 1)
                tp = transpose_packets(g + 1) if g + 1 < NG else iter(())
                cp = context_packets(g - 1) if g >= 1 else iter(())
                done = False
                while not done:
                    done = True
                    for _ in range(8):
                        pkt = next(tp, None)
                        if pkt is not None:
                            pkt()
                            done = False
                    pkt = next(cp, None)
                    if pkt is not None:
                        pkt()
                        done = False
                issue_logits_softmax(g)
            issue_alpha_t(NG - 1)
            issue_context(NG - 1)

    if not nc.is_finalized():
        nc.finalize()
    return nc


def _get_nc():
    if "nc" not in _CACHE:
        _CACHE["nc"] = build()
    return _CACHE["nc"]


def _install_ntff_hook():
    """The agent image's antenv lacks axon_hooks, so bass_utils' trace path
    can't find the NTFF profile hook. Recreate the module and install the
    ctypes-based hook from trn_agent_boot."""
    import types

    try:
        import antenv.axon_hooks  # noqa: F401
        return True
    except ImportError:
        pass
    try:
        import antenv
        from trn_agent_boot.trn_boot import _ntff_profile_via_ctypes

        hook = _ntff_profile_via_ctypes("/opt/axon/libaxon_pjrt.so")
        mod = types.ModuleType("antenv.axon_hooks")
        mod._hook = hook
        mod.set_axon_ntff_profile_hook = lambda h: setattr(mod, "_hook", h)
        mod.get_axon_ntff_profile_hook = lambda: mod._hook
        sys.modules["antenv.axon_hooks"] = mod
        antenv.axon_hooks = mod
        return hook is not None
    except Exception as e:  # pragma: no cover
        print(f"ntff hook install failed: {e}")
        return False


def run(inputs, trace=False):
    if trace:
        _install_ntff_hook()
    nc = _get_nc()
    enc = np.ascontiguousarray(inputs["enc_out"], dtype=np.float32)
    dec = np.ascontiguousarray(inputs["dec_h"], dtype=np.float32)
    shared = {
        "W_enc": np.ascontiguousarray(inputs["W_enc"], dtype=np.float32),
        "b_enc": np.ascontiguousarray(inputs["b_enc"], dtype=np.float32),
        "W_dec": np.ascontiguousarray(inputs["W_dec"], dtype=np.float32),
        "b_dec": np.ascontiguousarray(inputs["b_dec"], dtype=np.float32),
        "W_v": np.ascontiguousarray(inputs["W_v"], dtype=np.float32),
    }
    in_maps = []
    for i in range(N_CORES):
        m = dict(shared)
        m["enc_out"] = enc[i * BL : (i + 1) * BL]
        m["dec_h"] = dec[i * BL : (i + 1) * BL]
        in_maps.append(m)
    res = run_bass_kernel_spmd(nc, in_maps, core_ids=list(range(N_CORES)), trace=trace)
    outs = [res.results[i]["out"] for i in range(N_CORES)]
    full = np.concatenate(outs, axis=0)  # [512, 2244]
    context = np.ascontiguousarray(full[:, :ENC])
    alpha = np.ascontiguousarray(full[:, ENC:])
    return (context, alpha), res


def kernel(**inputs):
    (context, alpha), _ = run(inputs, trace=False)
    return (context, alpha)


# revision 36
# speedup vs baseline: 1.0555x; 1.0555x over previous
"""Trainium2 Bass kernel for Bahdanau-style attention (nn_Attention).

Computation (per batch b):
  attn1 = enc_out @ W_enc + b_enc          # [HW, ATTN]
  attn2 = dec_h @ W_dec + b_dec            # [ATTN]
  score = relu(attn1 + attn2)              # [HW, ATTN]
  logits = score @ W_v (+ b_v)             # [HW]  (b_v dropped: softmax-invariant)
  alpha = softmax(logits)                  # [HW]
  context = alpha @ enc_out                # [ENC]
Returns (context [B, ENC] f32, alpha [B, HW] f32).

Strategy: pure data-parallel over batch across 8 NeuronCores (64 batches/core),
batches processed in pipelined groups of 4:
  - enc tiles cast-DMA'd HBM f32 -> SBUF bf16 natural layout (SWDGE cast),
    as contiguous group-row tiles (6x128 + 1x16 rows per group).
  - encT (the moving operand of the big matmul) built by PE transposes written
    as REGULAR identity matmuls (exact, and unlike is_transpose they count as
    PE activity for the HAM clock gate); PSUM f32 evacuated with an fp8 cast
    split across DVE/ACT.
  - attn1^T = W_enc.T @ enc.T in fp8e4m3 with DoubleRow perf mode (2 e-chunks
    contracted per pass via the 3D-AP pair form); ~2x PE throughput vs bf16 at
    ~1.3e-2 final rel err (vs 2.5e-3 all-bf16).
  - bias (b_enc + b_dec + attn2_b) folded into the PSUM evacuation on ACT
    (per-partition bias + relu + cast to bf16 score).
  - logits via W_v-stationary matmuls; each batch's logits land on PSUM
    partition 32*j via tile_position, so softmax runs batched on one tile
    (reduce_max(negate) -> Exp with bias and fused accum_out -> reciprocal).
  - alpha transposed back to columns by one PE matmul per row-half; zero-padded
    block-diagonal A tiles kill cross-batch terms so the context accumulates a
    whole group in one PSUM group per 512-col chunk.
  - Issue order pipelines groups: loads lead by 2 groups; the LDW-heavy
    transpose packets for group g+1 are interleaved with the dense context
    matmuls of group g-1 to keep the HAM busy-fraction up; attn1 stays a dense
    fp8 block.
"""

import sys

if "/opt/trn_rl_repo" not in sys.path:
    sys.path.insert(0, "/opt/trn_rl_repo")

import numpy as np

import concourse.bass as bass
import concourse.tile as tile
from concourse import bacc, mybir
from concourse.bass_utils import run_bass_kernel_spmd
from concourse.masks import make_identity

N_CORES = 8
B, HW, ENC, DEC, ATTN = 512, 196, 2048, 512, 512
BL = B // N_CORES  # 64 batches per core
G = 4              # batches per group
NG = BL // G       # 16 groups
HW0 = 128
HW1 = HW - HW0     # 68
OUTW = ENC + HW    # context (2048) + alpha (196)

FP32 = mybir.dt.float32
BF16 = mybir.dt.bfloat16
FP8 = mybir.dt.float8e4
DR = mybir.MatmulPerfMode.DoubleRow
AX = mybir.AxisListType.X
AF = mybir.ActivationFunctionType

_CACHE = {}


def build():
    from contextlib import ExitStack

    nc = bacc.Bacc(
        "TRN2", target_bir_lowering=False, debug=False, num_devices=N_CORES
    )
    enc_d = nc.declare_dram_parameter("enc_out", [BL, HW, ENC], FP32, isOutput=False)
    dec_d = nc.declare_dram_parameter("dec_h", [BL, DEC], FP32, isOutput=False)
    wenc_d = nc.declare_dram_parameter("W_enc", [ENC, ATTN], FP32, isOutput=False)
    benc_d = nc.declare_dram_parameter("b_enc", [ATTN], FP32, isOutput=False)
    wdec_d = nc.declare_dram_parameter("W_dec", [DEC, ATTN], FP32, isOutput=False)
    bdec_d = nc.declare_dram_parameter("b_dec", [ATTN], FP32, isOutput=False)
    wv_d = nc.declare_dram_parameter("W_v", [ATTN], FP32, isOutput=False)
    out_d = nc.declare_dram_parameter("out", [BL, OUTW], FP32, isOutput=True)

    with tile.TileContext(nc) as tc:
        with ExitStack() as ctx:
            singles = ctx.enter_context(tc.tile_pool(name="singles", bufs=1))

            ident_bf = singles.tile([128, 128], BF16)
            make_identity(nc, ident_bf)

            # per-ATTN-chunk column layouts of the small vectors
            benc_t = singles.tile([128, 4], FP32)
            nc.gpsimd.dma_start(
                out=benc_t, in_=benc_d.rearrange("(ac p) -> p ac", p=128)
            )
            bdec_t = singles.tile([128, 4], FP32)
            nc.gpsimd.dma_start(
                out=bdec_t, in_=bdec_d.rearrange("(ac p) -> p ac", p=128)
            )
            bias_vec = singles.tile([128, 4], FP32)
            nc.vector.tensor_add(bias_vec, benc_t, bdec_t)

            wv_f = singles.tile([128, 4], FP32)
            nc.gpsimd.dma_start(out=wv_f, in_=wv_d.rearrange("(ac p) -> p ac", p=128))
            wv_bf = singles.tile([128, 4], BF16)
            nc.vector.tensor_copy(wv_bf, wv_f)

            # ---- attn2 / bias precompute: biasT[:, ac, b] = (dec_h @ W_dec + b_dec + b_enc)^T
            biasT = singles.tile([128, 4, BL], FP32)
            with (
                tc.tile_pool(name="pre", bufs=1) as pre,
                tc.tile_pool(name="pre_ps", bufs=2, space="PSUM") as pre_ps,
            ):
                dec_sb = pre.tile([BL, DEC], BF16)
                nc.gpsimd.dma_start(out=dec_sb, in_=dec_d[:, :])
                wdec_sb = pre.tile([128, 4, ATTN], BF16)
                for dc in range(4):
                    nc.gpsimd.dma_start(
                        out=wdec_sb[:, dc, :], in_=wdec_d[dc * 128 : (dc + 1) * 128, :]
                    )
                dechT = pre.tile([128, 4, BL], BF16)
                for dc in range(4):
                    ps = pre_ps.tile([128, BL], FP32, tag="prepst")
                    nc.tensor.matmul(
                        ps,
                        dec_sb[:, dc * 128 : (dc + 1) * 128],
                        ident_bf[0:BL, 0:BL],
                        start=True,
                        stop=True,
                    )
                    nc.vector.tensor_copy(dechT[:, dc, :], ps)
                for ac in range(4):
                    ps2 = pre_ps.tile([128, BL], FP32, tag="preps")
                    for dc in range(4):
                        nc.tensor.matmul(
                            ps2,
                            wdec_sb[:, dc, ac * 128 : (ac + 1) * 128],
                            dechT[:, dc, :],
                            start=(dc == 0),
                            stop=(dc == 3),
                        )
                    nc.vector.tensor_scalar_add(
                        biasT[:, ac, :], ps2, bias_vec[:, ac : ac + 1]
                    )

            # ---- main pools
            nat_p = ctx.enter_context(tc.tile_pool(name="nat", bufs=18))
            natS_p = ctx.enter_context(tc.tile_pool(name="natS", bufs=3))
            enct_p = ctx.enter_context(tc.tile_pool(name="enct", bufs=2))
            score_p = ctx.enter_context(tc.tile_pool(name="score", bufs=2))
            smax_p = ctx.enter_context(tc.tile_pool(name="smax", bufs=2))
            outs_p = ctx.enter_context(tc.tile_pool(name="outs", bufs=2))
            ps_a = ctx.enter_context(tc.tile_pool(name="ps_a", bufs=3, space="PSUM"))
            ps_t = ctx.enter_context(tc.tile_pool(name="ps_t", bufs=3, space="PSUM"))
            ps_c_p = ctx.enter_context(tc.tile_pool(name="ps_c", bufs=2, space="PSUM"))

            nat_tiles = {}
            enct_tiles = {}
            score_tiles = {}
            a_tiles = {}
            alpha_tiles = {}

            GR = G * HW          # 784 rows per group
            NT_FULL = GR // 128  # 6 full tiles
            TAIL = GR - NT_FULL * 128  # 16
            TSIZES = [128] * NT_FULL + [TAIL]
            encF = enc_d.rearrange("b s e -> (b s) e")

            def issue_loads(g):
                tiles = []
                for t, p in enumerate(TSIZES):
                    pool = nat_p if p == 128 else natS_p
                    nt = pool.tile([p, ENC], BF16, tag=f"nat{'S' if p < 128 else '0'}")
                    r0 = g * GR + t * 128
                    nc.gpsimd.dma_start(out=nt, in_=encF[r0 : r0 + p, :])
                    tiles.append(nt)
                nat_tiles[g] = tiles

            def transpose_packets(g):
                """Yield closures: 4 transpose matmuls + 1 evac each."""
                enct = enct_p.tile([128, 16, G * HW], FP8)
                enct_tiles[g] = enct

                def pkt(t, kq):
                    nt = nat_tiles[g][t]
                    p = TSIZES[t]
                    ps0 = ps_t.tile([128, 4 * p], FP32, tag="pst")
                    for u in range(4):
                        kc = kq * 4 + u
                        nc.tensor.matmul(
                            ps0[:, u * p : (u + 1) * p],
                            nt[:, kc * 128 : (kc + 1) * 128],
                            ident_bf[0:p, 0:p],
                            start=True,
                            stop=True,
                        )
                    ev = nc.scalar.copy if t in (1, 4) else nc.vector.tensor_copy
                    ev(
                        enct[:, kq * 4 : (kq + 1) * 4, t * 128 : t * 128 + p],
                        ps0.rearrange("p (u c) -> p u c", u=4),
                    )

                for t in range(len(TSIZES)):
                    for kq in range(4):
                        yield lambda t=t, kq=kq: pkt(t, kq)

            def issue_transpose(g):
                for pkt in transpose_packets(g):
                    pkt()

            def issue_attn1(g):
                enct = enct_tiles[g]
                sco = score_p.tile([128, 4, G * HW], BF16, tag="score")
                score_tiles[g] = sco
                half_n = G * HW // 2  # 392 = 2 batches
                for ac in range(4):
                    for half in range(2):
                        ps = ps_a.tile([128, half_n], FP32, tag="psa")
                        for kc2 in range(8):
                            nc.tensor.matmul(
                                ps,
                                w8[:, 2 * kc2 : 2 * kc2 + 2, ac * 128 : (ac + 1) * 128],
                                enct[:, 2 * kc2 : 2 * kc2 + 2, half * half_n : (half + 1) * half_n],
                                start=(kc2 == 0),
                                stop=(kc2 == 7),
                                perf_mode=DR,
                            )
                        for j2 in range(2):
                            j = half * 2 + j2
                            b = g * G + j
                            nc.scalar.activation(
                                out=sco[:, ac, j * HW : (j + 1) * HW],
                                in_=ps[:, j2 * HW : (j2 + 1) * HW],
                                func=AF.Relu,
                                bias=biasT[:, ac, b : b + 1],
                            )

            def issue_logits_softmax(g):
                sco = score_tiles[g]
                ps_lg = ps_c_p.tile([97, HW], FP32, tag="psc")
                for j in range(G):
                    for ac in range(4):
                        nc.tensor.matmul(
                            ps_lg[32 * j : 32 * j + 1, :],
                            wv_bf[:, ac : ac + 1],
                            sco[:, ac, j * HW : (j + 1) * HW],
                            start=(ac == 0),
                            stop=(ac == 3),
                            tile_position=(0, 32 * j),
                        )
                st = smax_p.tile([97, 4], FP32, tag="smx")
                ex = smax_p.tile([97, HW], FP32, tag="ex")
                alpha_full = smax_p.tile([97, HW], FP32, tag="alpha")
                alpha_tiles[g] = alpha_full
                nc.vector.reduce_max(st[:, 0:1], ps_lg, axis=AX, negate=True)
                nc.scalar.activation(
                    out=ex,
                    in_=ps_lg,
                    func=AF.Exp,
                    bias=st[:, 0:1],
                    accum_out=st[:, 1:2],
                )
                nc.vector.reciprocal(st[:, 2:3], st[:, 1:2])
                nc.vector.tensor_scalar_mul(alpha_full, ex, st[:, 2:3])
                for j in range(G):
                    bg = g * G + j
                    nc.sync.dma_start(
                        out=out_d[bg : bg + 1, ENC : ENC + HW],
                        in_=alpha_full[32 * j : 32 * j + 1, :],
                    )

            def issue_alpha_t(g):
                alpha_full = alpha_tiles[g]
                # diagonal alpha at partitions {0,32,64,96}: af[32j, r] =
                # alpha_j(r - j*HW) within batch j's row range, else 0
                af = smax_p.tile([97, GR], BF16, tag="aflat")
                nc.vector.memset(af, 0.0)
                for j in range(G):
                    nc.vector.tensor_copy(
                        af[32 * j : 32 * j + 1, j * HW : (j + 1) * HW],
                        alpha_full[32 * j : 32 * j + 1, :],
                    )
                a_sb = smax_p.tile([128, len(TSIZES), G], BF16, tag="asb")
                a_tiles[g] = a_sb
                for t, p in enumerate(TSIZES):
                    ps_at = ps_c_p.tile([128, 98], FP32, tag="psc")
                    nc.tensor.matmul(
                        ps_at[0:p, 0:97],
                        af[:, t * 128 : t * 128 + p],
                        ident_bf[0:97, 0:97],
                        start=True,
                        stop=True,
                    )
                    nc.vector.tensor_copy(a_sb[0:p, t, :], ps_at[0:p, 0:97:32])

            def context_packets(g):
                a_sb = a_tiles[g]
                ctx_sb = outs_p.tile([G, ENC], FP32, tag="ctx")

                def nchunk(nch):
                    ps_c = ps_c_p.tile([G, 512], FP32, tag="psc")
                    for t, p in enumerate(TSIZES):
                        nc.tensor.matmul(
                            ps_c,
                            a_sb[0:p, t, :],
                            nat_tiles[g][t][:, nch * 512 : (nch + 1) * 512],
                            start=(t == 0),
                            stop=(t == len(TSIZES) - 1),
                        )
                    nc.scalar.copy(ctx_sb[:, nch * 512 : (nch + 1) * 512], ps_c)

                for nch in range(4):
                    yield lambda nch=nch: nchunk(nch)
                yield lambda: nc.sync.dma_start(
                    out=out_d[g * G : (g + 1) * G, 0:ENC], in_=ctx_sb
                )

            def issue_context(g):
                for pkt in context_packets(g):
                    pkt()

            issue_loads(0)
            # W_enc chunks: bf16 cast-DMA load, then fp8 copy for DoubleRow matmuls
            w_bf = singles.tile([128, 16, ATTN], BF16)
            w8 = singles.tile([128, 16, ATTN], FP8)
            for kc in range(16):
                nc.gpsimd.dma_start(
                    out=w_bf[:, kc, :], in_=wenc_d[kc * 128 : (kc + 1) * 128, :]
                )
                nc.vector.tensor_copy(w8[:, kc, :], w_bf[:, kc, :])

            issue_loads(1)
            issue_transpose(0)
            for g in range(NG):
                if g + 2 < NG:
                    issue_loads(g + 2)
                issue_attn1(g)
                if g >= 1:
                    issue_alpha_t(g - 1)
                tp = transpose_packets(g + 1) if g + 1 < NG else iter(())
                cp = context_packets(g - 1) if g >= 1 else iter(())
                done = False
                while not done:
                    done = True
                    for _ in range(8):
                        pkt = next(tp, None)
                        if pkt is not None:
                            pkt()
                            done = False
                    pkt = next(cp, None)
                    if pkt is not None:
                        pkt()
                        done = False
                issue_logits_softmax(g)
            issue_alpha_t(NG - 1)
            issue_context(NG - 1)

    if not nc.is_finalized():
        nc.finalize()
    return nc


def _get_nc():
    if "nc" not in _CACHE:
        _CACHE["nc"] = build()
    return _CACHE["nc"]


def _install_ntff_hook():
    """The agent image's antenv lacks axon_hooks, so bass_utils' trace path
    can't find the NTFF profile hook. Recreate the module and install the
    ctypes-based hook from trn_agent_boot."""
    import types

    try:
        import antenv.axon_hooks  # noqa: F401
        return True
    except ImportError:
        pass
    try:
        import antenv
        from trn_agent_boot.trn_boot import _ntff_profile_via_ctypes

        hook = _ntff_profile_via_ctypes("/opt/axon/libaxon_pjrt.so")
        mod = types.ModuleType("antenv.axon_hooks")
        mod._hook = hook
        mod.set_axon_ntff_profile_hook = lambda h: setattr(mod, "_hook", h)
        mod.get_axon_ntff_profile_hook = lambda: mod._hook
        sys.modules["antenv.axon_hooks"] = mod
        antenv.axon_hooks = mod
        return hook is not None
    except Exception as e:  # pragma: no cover
        print(f"ntff hook install failed: {e}")
        return False


def run(inputs, trace=False):
    if trace:
        _install_ntff_hook()
    nc = _get_nc()
    enc = np.ascontiguousarray(inputs["enc_out"], dtype=np.float32)
    dec = np.ascontiguousarray(inputs["dec_h"], dtype=np.float32)
    shared = {
        "W_enc": np.ascontiguousarray(inputs["W_enc"], dtype=np.float32),
        "b_enc": np.ascontiguousarray(inputs["b_enc"], dtype=np.float32),
        "W_dec": np.ascontiguousarray(inputs["W_dec"], dtype=np.float32),
        "b_dec": np.ascontiguousarray(inputs["b_dec"], dtype=np.float32),
        "W_v": np.ascontiguousarray(inputs["W_v"], dtype=np.float32),
    }
    in_maps = []
    for i in range(N_CORES):
        m = dict(shared)
        m["enc_out"] = enc[i * BL : (i + 1) * BL]
        m["dec_h"] = dec[i * BL : (i + 1) * BL]
        in_maps.append(m)
    res = run_bass_kernel_spmd(nc, in_maps, core_ids=list(range(N_CORES)), trace=trace)
    outs = [res.results[i]["out"] for i in range(N_CORES)]
    full = np.concatenate(outs, axis=0)  # [512, 2244]
    context = np.ascontiguousarray(full[:, :ENC])
    alpha = np.ascontiguousarray(full[:, ENC:])
    return (context, alpha), res


def kernel(**inputs):
    (context, alpha), _ = run(inputs, trace=False)
    return (context, alpha)


# revision 37
# speedup vs baseline: 1.0770x; 1.0204x over previous
"""Trainium2 Bass kernel for Bahdanau-style attention (nn_Attention).

Computation (per batch b):
  attn1 = enc_out @ W_enc + b_enc          # [HW, ATTN]
  attn2 = dec_h @ W_dec + b_dec            # [ATTN]
  score = relu(attn1 + attn2)              # [HW, ATTN]
  logits = score @ W_v (+ b_v)             # [HW]  (b_v dropped: softmax-invariant)
  alpha = softmax(logits)                  # [HW]
  context = alpha @ enc_out                # [ENC]
Returns (context [B, ENC] f32, alpha [B, HW] f32).

Strategy: pure data-parallel over batch across 8 NeuronCores (64 batches/core),
batches processed in pipelined groups of 4:
  - enc tiles cast-DMA'd HBM f32 -> SBUF bf16 natural layout (SWDGE cast),
    as contiguous group-row tiles (6x128 + 1x16 rows per group).
  - encT (the moving operand of the big matmul) built by PE transposes written
    as REGULAR identity matmuls (exact, and unlike is_transpose they count as
    PE activity for the HAM clock gate); PSUM f32 evacuated with an fp8 cast
    split across DVE/ACT.
  - attn1^T = W_enc.T @ enc.T in fp8e4m3 with DoubleRow perf mode (2 e-chunks
    contracted per pass via the 3D-AP pair form); ~2x PE throughput vs bf16 at
    ~1.3e-2 final rel err (vs 2.5e-3 all-bf16).
  - bias (b_enc + b_dec + attn2_b) folded into the PSUM evacuation on ACT
    (per-partition bias + relu + cast to bf16 score).
  - logits via W_v-stationary matmuls; each batch's logits land on PSUM
    partition 32*j via tile_position, so softmax runs batched on one tile
    (reduce_max(negate) -> Exp with bias and fused accum_out -> reciprocal).
  - alpha transposed back to columns by one PE matmul per row-half; zero-padded
    block-diagonal A tiles kill cross-batch terms so the context accumulates a
    whole group in one PSUM group per 512-col chunk.
  - Issue order pipelines groups: loads lead by 2 groups; the LDW-heavy
    transpose packets for group g+1 are interleaved with the dense context
    matmuls of group g-1 to keep the HAM busy-fraction up; attn1 stays a dense
    fp8 block.
"""

import sys

if "/opt/trn_rl_repo" not in sys.path:
    sys.path.insert(0, "/opt/trn_rl_repo")

import numpy as np

import concourse.bass as bass
import concourse.tile as tile
from concourse import bacc, mybir
from concourse.bass_utils import run_bass_kernel_spmd
from concourse.masks import make_identity

N_CORES = 8
B, HW, ENC, DEC, ATTN = 512, 196, 2048, 512, 512
BL = B // N_CORES  # 64 batches per core
G = 4              # batches per group
NG = BL // G       # 16 groups
HW0 = 128
HW1 = HW - HW0     # 68
OUTW = ENC + HW    # context (2048) + alpha (196)

FP32 = mybir.dt.float32
BF16 = mybir.dt.bfloat16
FP8 = mybir.dt.float8e4
DR = mybir.MatmulPerfMode.DoubleRow
AX = mybir.AxisListType.X
AF = mybir.ActivationFunctionType

_CACHE = {}


def build():
    from contextlib import ExitStack

    nc = bacc.Bacc(
        "TRN2", target_bir_lowering=False, debug=False, num_devices=N_CORES
    )
    enc_d = nc.declare_dram_parameter("enc_out", [BL, HW, ENC], FP32, isOutput=False)
    dec_d = nc.declare_dram_parameter("dec_h", [BL, DEC], FP32, isOutput=False)
    wenc_d = nc.declare_dram_parameter("W_enc", [ENC, ATTN], FP32, isOutput=False)
    benc_d = nc.declare_dram_parameter("b_enc", [ATTN], FP32, isOutput=False)
    wdec_d = nc.declare_dram_parameter("W_dec", [DEC, ATTN], FP32, isOutput=False)
    bdec_d = nc.declare_dram_parameter("b_dec", [ATTN], FP32, isOutput=False)
    wv_d = nc.declare_dram_parameter("W_v", [ATTN], FP32, isOutput=False)
    out_d = nc.declare_dram_parameter("out", [BL, OUTW], FP32, isOutput=True)

    with tile.TileContext(nc) as tc:
        with ExitStack() as ctx:
            singles = ctx.enter_context(tc.tile_pool(name="singles", bufs=1))

            ident_bf = singles.tile([128, 128], BF16)
            make_identity(nc, ident_bf)

            # per-ATTN-chunk column layouts of the small vectors
            benc_t = singles.tile([128, 4], FP32)
            nc.gpsimd.dma_start(
                out=benc_t, in_=benc_d.rearrange("(ac p) -> p ac", p=128)
            )
            bdec_t = singles.tile([128, 4], FP32)
            nc.gpsimd.dma_start(
                out=bdec_t, in_=bdec_d.rearrange("(ac p) -> p ac", p=128)
            )
            bias_vec = singles.tile([128, 4], FP32)
            nc.vector.tensor_add(bias_vec, benc_t, bdec_t)

            wv_f = singles.tile([128, 4], FP32)
            nc.gpsimd.dma_start(out=wv_f, in_=wv_d.rearrange("(ac p) -> p ac", p=128))
            wv_bf = singles.tile([128, 4], BF16)
            nc.vector.tensor_copy(wv_bf, wv_f)

            # ---- attn2 / bias precompute: biasT[:, ac, b] = (dec_h @ W_dec + b_dec + b_enc)^T
            biasT = singles.tile([128, 4, BL], FP32)
            with (
                tc.tile_pool(name="pre", bufs=1) as pre,
                tc.tile_pool(name="pre_ps", bufs=2, space="PSUM") as pre_ps,
            ):
                dec_sb = pre.tile([BL, DEC], BF16)
                nc.gpsimd.dma_start(out=dec_sb, in_=dec_d[:, :])
                wdec_sb = pre.tile([128, 4, ATTN], BF16)
                for dc in range(4):
                    nc.gpsimd.dma_start(
                        out=wdec_sb[:, dc, :], in_=wdec_d[dc * 128 : (dc + 1) * 128, :]
                    )
                dechT = pre.tile([128, 4, BL], BF16)
                for dc in range(4):
                    ps = pre_ps.tile([128, BL], FP32, tag="prepst")
                    nc.tensor.matmul(
                        ps,
                        dec_sb[:, dc * 128 : (dc + 1) * 128],
                        ident_bf[0:BL, 0:BL],
                        start=True,
                        stop=True,
                    )
                    nc.vector.tensor_copy(dechT[:, dc, :], ps)
                for ac in range(4):
                    ps2 = pre_ps.tile([128, BL], FP32, tag="preps")
                    for dc in range(4):
                        nc.tensor.matmul(
                            ps2,
                            wdec_sb[:, dc, ac * 128 : (ac + 1) * 128],
                            dechT[:, dc, :],
                            start=(dc == 0),
                            stop=(dc == 3),
                        )
                    nc.vector.tensor_scalar_add(
                        biasT[:, ac, :], ps2, bias_vec[:, ac : ac + 1]
                    )

            # ---- main pools
            nat_p = ctx.enter_context(tc.tile_pool(name="nat", bufs=18))
            natS_p = ctx.enter_context(tc.tile_pool(name="natS", bufs=3))
            enct_p = ctx.enter_context(tc.tile_pool(name="enct", bufs=2))
            score_p = ctx.enter_context(tc.tile_pool(name="score", bufs=2))
            smax_p = ctx.enter_context(tc.tile_pool(name="smax", bufs=2))
            outs_p = ctx.enter_context(tc.tile_pool(name="outs", bufs=2))
            ps_a = ctx.enter_context(tc.tile_pool(name="ps_a", bufs=2, space="PSUM"))
            ps_t = ctx.enter_context(tc.tile_pool(name="ps_t", bufs=4, space="PSUM"))
            ps_c_p = ctx.enter_context(tc.tile_pool(name="ps_c", bufs=2, space="PSUM"))

            nat_tiles = {}
            enct_tiles = {}
            score_tiles = {}
            a_tiles = {}
            alpha_tiles = {}

            GR = G * HW          # 784 rows per group
            NT_FULL = GR // 128  # 6 full tiles
            TAIL = GR - NT_FULL * 128  # 16
            TSIZES = [128] * NT_FULL + [TAIL]
            encF = enc_d.rearrange("b s e -> (b s) e")

            def issue_loads(g):
                tiles = []
                for t, p in enumerate(TSIZES):
                    pool = nat_p if p == 128 else natS_p
                    nt = pool.tile([p, ENC], BF16, tag=f"nat{'S' if p < 128 else '0'}")
                    r0 = g * GR + t * 128
                    nc.gpsimd.dma_start(out=nt, in_=encF[r0 : r0 + p, :])
                    tiles.append(nt)
                nat_tiles[g] = tiles

            def transpose_packets(g):
                """Yield closures: 4 transpose matmuls + 1 evac each."""
                enct = enct_p.tile([128, 16, G * HW], FP8)
                enct_tiles[g] = enct

                def pkt(t, kq):
                    nt = nat_tiles[g][t]
                    p = TSIZES[t]
                    ps0 = ps_t.tile([128, 4 * p], FP32, tag="pst")
                    for u in range(4):
                        kc = kq * 4 + u
                        nc.tensor.matmul(
                            ps0[:, u * p : (u + 1) * p],
                            nt[:, kc * 128 : (kc + 1) * 128],
                            ident_bf[0:p, 0:p],
                            start=True,
                            stop=True,
                        )
                    ev = nc.scalar.copy if t in (1, 4) else nc.vector.tensor_copy
                    ev(
                        enct[:, kq * 4 : (kq + 1) * 4, t * 128 : t * 128 + p],
                        ps0.rearrange("p (u c) -> p u c", u=4),
                    )

                for t in range(len(TSIZES)):
                    for kq in range(4):
                        yield lambda t=t, kq=kq: pkt(t, kq)

            def issue_transpose(g):
                for pkt in transpose_packets(g):
                    pkt()

            def issue_attn1(g):
                enct = enct_tiles[g]
                sco = score_p.tile([128, 4, G * HW], BF16, tag="score")
                score_tiles[g] = sco
                half_n = G * HW // 2  # 392 = 2 batches
                for ac in range(4):
                    for half in range(2):
                        ps = ps_a.tile([128, half_n], FP32, tag="psa")
                        for kc2 in range(8):
                            nc.tensor.matmul(
                                ps,
                                w8[:, 2 * kc2 : 2 * kc2 + 2, ac * 128 : (ac + 1) * 128],
                                enct[:, 2 * kc2 : 2 * kc2 + 2, half * half_n : (half + 1) * half_n],
                                start=(kc2 == 0),
                                stop=(kc2 == 7),
                                perf_mode=DR,
                            )
                        for j2 in range(2):
                            j = half * 2 + j2
                            b = g * G + j
                            nc.scalar.activation(
                                out=sco[:, ac, j * HW : (j + 1) * HW],
                                in_=ps[:, j2 * HW : (j2 + 1) * HW],
                                func=AF.Relu,
                                bias=biasT[:, ac, b : b + 1],
                            )

            def issue_logits_softmax(g):
                sco = score_tiles[g]
                ps_lg = ps_c_p.tile([97, HW], FP32, tag="psc")
                for j in range(G):
                    for ac in range(4):
                        nc.tensor.matmul(
                            ps_lg[32 * j : 32 * j + 1, :],
                            wv_bf[:, ac : ac + 1],
                            sco[:, ac, j * HW : (j + 1) * HW],
                            start=(ac == 0),
                            stop=(ac == 3),
                            tile_position=(0, 32 * j),
                        )
                st = smax_p.tile([97, 4], FP32, tag="smx")
                ex = smax_p.tile([97, HW], FP32, tag="ex")
                alpha_full = smax_p.tile([97, HW], FP32, tag="alpha")
                alpha_tiles[g] = alpha_full
                nc.vector.reduce_max(st[:, 0:1], ps_lg, axis=AX, negate=True)
                nc.scalar.activation(
                    out=ex,
                    in_=ps_lg,
                    func=AF.Exp,
                    bias=st[:, 0:1],
                    accum_out=st[:, 1:2],
                )
                nc.vector.reciprocal(st[:, 2:3], st[:, 1:2])
                nc.vector.tensor_scalar_mul(alpha_full, ex, st[:, 2:3])
                for j in range(G):
                    bg = g * G + j
                    nc.sync.dma_start(
                        out=out_d[bg : bg + 1, ENC : ENC + HW],
                        in_=alpha_full[32 * j : 32 * j + 1, :],
                    )

            def issue_alpha_t(g):
                alpha_full = alpha_tiles[g]
                # diagonal alpha at partitions {0,32,64,96}: af[32j, r] =
                # alpha_j(r - j*HW) within batch j's row range, else 0
                af = smax_p.tile([97, GR], BF16, tag="aflat")
                nc.vector.memset(af, 0.0)
                for j in range(G):
                    nc.vector.tensor_copy(
                        af[32 * j : 32 * j + 1, j * HW : (j + 1) * HW],
                        alpha_full[32 * j : 32 * j + 1, :],
                    )
                a_sb = smax_p.tile([128, len(TSIZES), G], BF16, tag="asb")
                a_tiles[g] = a_sb
                for t, p in enumerate(TSIZES):
                    ps_at = ps_c_p.tile([128, 98], FP32, tag="psc")
                    nc.tensor.matmul(
                        ps_at[0:p, 0:97],
                        af[:, t * 128 : t * 128 + p],
                        ident_bf[0:97, 0:97],
                        start=True,
                        stop=True,
                    )
                    nc.vector.tensor_copy(a_sb[0:p, t, :], ps_at[0:p, 0:97:32])

            def context_packets(g):
                a_sb = a_tiles[g]
                ctx_sb = outs_p.tile([G, ENC], FP32, tag="ctx")

                def nchunk(nch):
                    ps_c = ps_c_p.tile([G, 512], FP32, tag="psc")
                    for t, p in enumerate(TSIZES):
                        nc.tensor.matmul(
                            ps_c,
                            a_sb[0:p, t, :],
                            nat_tiles[g][t][:, nch * 512 : (nch + 1) * 512],
                            start=(t == 0),
                            stop=(t == len(TSIZES) - 1),
                        )
                    nc.scalar.copy(ctx_sb[:, nch * 512 : (nch + 1) * 512], ps_c)

                for nch in range(4):
                    yield lambda nch=nch: nchunk(nch)
                yield lambda: nc.sync.dma_start(
                    out=out_d[g * G : (g + 1) * G, 0:ENC], in_=ctx_sb
                )

            def issue_context(g):
                for pkt in context_packets(g):
                    pkt()

            issue_loads(0)
            # W_enc chunks: bf16 cast-DMA load, then fp8 copy for DoubleRow matmuls
            w_bf = singles.tile([128, 16, ATTN], BF16)
            w8 = singles.tile([128, 16, ATTN], FP8)
            for kc in range(16):
                nc.gpsimd.dma_start(
                    out=w_bf[:, kc, :], in_=wenc_d[kc * 128 : (kc + 1) * 128, :]
                )
                nc.vector.tensor_copy(w8[:, kc, :], w_bf[:, kc, :])

            issue_loads(1)
            issue_transpose(0)
            for g in range(NG):
                if g + 2 < NG:
                    issue_loads(g + 2)
                issue_attn1(g)
                if g >= 1:
                    issue_alpha_t(g - 1)
                tp = transpose_packets(g + 1) if g + 1 < NG else iter(())
                cp = context_packets(g - 1) if g >= 1 else iter(())
                done = False
                while not done:
                    done = True
                    for _ in range(8):
                        pkt = next(tp, None)
                        if pkt is not None:
                            pkt()
                            done = False
                    pkt = next(cp, None)
                    if pkt is not None:
                        pkt()
                        done = False
                issue_logits_softmax(g)
            issue_alpha_t(NG - 1)
            issue_context(NG - 1)

    if not nc.is_finalized():
        nc.finalize()
    return nc


def _get_nc():
    if "nc" not in _CACHE:
        _CACHE["nc"] = build()
    return _CACHE["nc"]


def _install_ntff_hook():
    """The agent image's antenv lacks axon_hooks, so bass_utils' trace path
    can't find the NTFF profile hook. Recreate the module and install the
    ctypes-based hook from trn_agent_boot."""
    import types

    try:
        import antenv.axon_hooks  # noqa: F401
        return True
    except ImportError:
        pass
    try:
        import antenv
        from trn_agent_boot.trn_boot import _ntff_profile_via_ctypes

        hook = _ntff_profile_via_ctypes("/opt/axon/libaxon_pjrt.so")
        mod = types.ModuleType("antenv.axon_hooks")
        mod._hook = hook
        mod.set_axon_ntff_profile_hook = lambda h: setattr(mod, "_hook", h)
        mod.get_axon_ntff_profile_hook = lambda: mod._hook
        sys.modules["antenv.axon_hooks"] = mod
        antenv.axon_hooks = mod
        return hook is not None
    except Exception as e:  # pragma: no cover
        print(f"ntff hook install failed: {e}")
        return False


def run(inputs, trace=False):
    if trace:
        _install_ntff_hook()
    nc = _get_nc()
    enc = np.ascontiguousarray(inputs["enc_out"], dtype=np.float32)
    dec = np.ascontiguousarray(inputs["dec_h"], dtype=np.float32)
    shared = {
        "W_enc": np.ascontiguousarray(inputs["W_enc"], dtype=np.float32),
        "b_enc": np.ascontiguousarray(inputs["b_enc"], dtype=np.float32),
        "W_dec": np.ascontiguousarray(inputs["W_dec"], dtype=np.float32),
        "b_dec": np.ascontiguousarray(inputs["b_dec"], dtype=np.float32),
        "W_v": np.ascontiguousarray(inputs["W_v"], dtype=np.float32),
    }
    in_maps = []
    for i in range(N_CORES):
        m = dict(shared)
        m["enc_out"] = enc[i * BL : (i + 1) * BL]
        m["dec_h"] = dec[i * BL : (i + 1) * BL]
        in_maps.append(m)
    res = run_bass_kernel_spmd(nc, in_maps, core_ids=list(range(N_CORES)), trace=trace)
    outs = [res.results[i]["out"] for i in range(N_CORES)]
    full = np.concatenate(outs, axis=0)  # [512, 2244]
    context = np.ascontiguousarray(full[:, :ENC])
    alpha = np.ascontiguousarray(full[:, ENC:])
    return (context, alpha), res


def kernel(**inputs):
    (context, alpha), _ = run(inputs, trace=False)
    return (context, alpha)


# revision 39
# speedup vs baseline: 1.0905x; 1.0125x over previous
"""Trainium2 Bass kernel for Bahdanau-style attention (nn_Attention).

Computation (per batch b):
  attn1 = enc_out @ W_enc + b_enc          # [HW, ATTN]
  attn2 = dec_h @ W_dec + b_dec            # [ATTN]
  score = relu(attn1 + attn2)              # [HW, ATTN]
  logits = score @ W_v (+ b_v)             # [HW]  (b_v dropped: softmax-invariant)
  alpha = softmax(logits)                  # [HW]
  context = alpha @ enc_out                # [ENC]
Returns (context [B, ENC] f32, alpha [B, HW] f32).

Strategy: pure data-parallel over batch across 8 NeuronCores (64 batches/core),
batches processed in pipelined groups of 4:
  - enc tiles cast-DMA'd HBM f32 -> SBUF bf16 natural layout (SWDGE cast),
    as contiguous group-row tiles (6x128 + 1x16 rows per group).
  - encT (the moving operand of the big matmul) built by PE transposes written
    as REGULAR identity matmuls (exact, and unlike is_transpose they count as
    PE activity for the HAM clock gate); PSUM f32 evacuated with an fp8 cast
    split across DVE/ACT.
  - attn1^T = W_enc.T @ enc.T in fp8e4m3 with DoubleRow perf mode (2 e-chunks
    contracted per pass via the 3D-AP pair form); ~2x PE throughput vs bf16 at
    ~1.3e-2 final rel err (vs 2.5e-3 all-bf16).
  - bias (b_enc + b_dec + attn2_b) folded into the PSUM evacuation on ACT
    (per-partition bias + relu + cast to bf16 score).
  - logits via W_v-stationary matmuls; each batch's logits land on PSUM
    partition 32*j via tile_position, so softmax runs batched on one tile
    (reduce_max(negate) -> Exp with bias and fused accum_out -> reciprocal).
  - alpha transposed back to columns by one PE matmul per row-half; zero-padded
    block-diagonal A tiles kill cross-batch terms so the context accumulates a
    whole group in one PSUM group per 512-col chunk.
  - Issue order pipelines groups: loads lead by 2 groups; the LDW-heavy
    transpose packets for group g+1 are interleaved with the dense context
    matmuls of group g-1 to keep the HAM busy-fraction up; attn1 stays a dense
    fp8 block.
"""

import sys

if "/opt/trn_rl_repo" not in sys.path:
    sys.path.insert(0, "/opt/trn_rl_repo")

import numpy as np

import concourse.bass as bass
import concourse.tile as tile
from concourse import bacc, mybir
from concourse.bass_utils import run_bass_kernel_spmd
from concourse.masks import make_identity

N_CORES = 8
B, HW, ENC, DEC, ATTN = 512, 196, 2048, 512, 512
BL = B // N_CORES  # 64 batches per core
G = 4              # batches per group
NG = BL // G       # 16 groups
HW0 = 128
HW1 = HW - HW0     # 68
OUTW = ENC + HW    # context (2048) + alpha (196)

FP32 = mybir.dt.float32
BF16 = mybir.dt.bfloat16
FP8 = mybir.dt.float8e4
DR = mybir.MatmulPerfMode.DoubleRow
AX = mybir.AxisListType.X
AF = mybir.ActivationFunctionType

_CACHE = {}


def build():
    from contextlib import ExitStack

    nc = bacc.Bacc(
        "TRN2", target_bir_lowering=False, debug=False, num_devices=N_CORES
    )
    enc_d = nc.declare_dram_parameter("enc_out", [BL, HW, ENC], FP32, isOutput=False)
    dec_d = nc.declare_dram_parameter("dec_h", [BL, DEC], FP32, isOutput=False)
    wenc_d = nc.declare_dram_parameter("W_enc", [ENC, ATTN], FP32, isOutput=False)
    benc_d = nc.declare_dram_parameter("b_enc", [ATTN], FP32, isOutput=False)
    wdec_d = nc.declare_dram_parameter("W_dec", [DEC, ATTN], FP32, isOutput=False)
    bdec_d = nc.declare_dram_parameter("b_dec", [ATTN], FP32, isOutput=False)
    wv_d = nc.declare_dram_parameter("W_v", [ATTN], FP32, isOutput=False)
    out_d = nc.declare_dram_parameter("out", [BL, OUTW], FP32, isOutput=True)

    with tile.TileContext(nc) as tc:
        with ExitStack() as ctx:
            singles = ctx.enter_context(tc.tile_pool(name="singles", bufs=1))

            ident_bf = singles.tile([128, 128], BF16)
            make_identity(nc, ident_bf)

            # ---- main pools
            nat_p = ctx.enter_context(tc.tile_pool(name="nat", bufs=18))
            natS_p = ctx.enter_context(tc.tile_pool(name="natS", bufs=3))
            enct_p = ctx.enter_context(tc.tile_pool(name="enct", bufs=2))
            score_p = ctx.enter_context(tc.tile_pool(name="score", bufs=2))
            smax_p = ctx.enter_context(tc.tile_pool(name="smax", bufs=2))
            outs_p = ctx.enter_context(tc.tile_pool(name="outs", bufs=2))
            ps_a = ctx.enter_context(tc.tile_pool(name="ps_a", bufs=2, space="PSUM"))
            ps_t = ctx.enter_context(tc.tile_pool(name="ps_t", bufs=4, space="PSUM"))
            ps_c_p = ctx.enter_context(tc.tile_pool(name="ps_c", bufs=2, space="PSUM"))

            nat_tiles = {}
            enct_tiles = {}
            score_tiles = {}
            a_tiles = {}
            alpha_tiles = {}

            GR = G * HW          # 784 rows per group
            NT_FULL = GR // 128  # 6 full tiles
            TAIL = GR - NT_FULL * 128  # 16
            TSIZES = [128] * NT_FULL + [TAIL]
            encF = enc_d.rearrange("b s e -> (b s) e")

            def issue_loads(g):
                tiles = []
                for t, p in enumerate(TSIZES):
                    pool = nat_p if p == 128 else natS_p
                    nt = pool.tile([p, ENC], BF16, tag=f"nat{'S' if p < 128 else '0'}")
                    r0 = g * GR + t * 128
                    nc.gpsimd.dma_start(out=nt, in_=encF[r0 : r0 + p, :])
                    tiles.append(nt)
                nat_tiles[g] = tiles

            def transpose_packets(g):
                """Yield closures: 4 transpose matmuls + 1 evac each."""
                enct = enct_p.tile([128, 16, G * HW], FP8)
                enct_tiles[g] = enct

                def pkt(t, kq):
                    nt = nat_tiles[g][t]
                    p = TSIZES[t]
                    ps0 = ps_t.tile([128, 4 * p], FP32, tag="pst")
                    for u in range(4):
                        kc = kq * 4 + u
                        nc.tensor.matmul(
                            ps0[:, u * p : (u + 1) * p],
                            nt[:, kc * 128 : (kc + 1) * 128],
                            ident_bf[0:p, 0:p],
                            start=True,
                            stop=True,
                        )
                    ev = nc.scalar.copy if t in (1, 4) else nc.vector.tensor_copy
                    ev(
                        enct[:, kq * 4 : (kq + 1) * 4, t * 128 : t * 128 + p],
                        ps0.rearrange("p (u c) -> p u c", u=4),
                    )

                for t in range(len(TSIZES)):
                    for kq in range(4):
                        yield lambda t=t, kq=kq: pkt(t, kq)

            def issue_transpose(g):
                for pkt in transpose_packets(g):
                    pkt()

            def issue_attn1(g):
                enct = enct_tiles[g]
                sco = score_p.tile([128, 4, G * HW], BF16, tag="score")
                score_tiles[g] = sco
                half_n = G * HW // 2  # 392 = 2 batches
                for ac in range(4):
                    for half in range(2):
                        ps = ps_a.tile([128, half_n], FP32, tag="psa")
                        for kc2 in range(8):
                            nc.tensor.matmul(
                                ps,
                                w8[:, 2 * kc2 : 2 * kc2 + 2, ac * 128 : (ac + 1) * 128],
                                enct[:, 2 * kc2 : 2 * kc2 + 2, half * half_n : (half + 1) * half_n],
                                start=(kc2 == 0),
                                stop=(kc2 == 7),
                                perf_mode=DR,
                            )
                        for j2 in range(2):
                            j = half * 2 + j2
                            b = g * G + j
                            nc.scalar.activation(
                                out=sco[:, ac, j * HW : (j + 1) * HW],
                                in_=ps[:, j2 * HW : (j2 + 1) * HW],
                                func=AF.Relu,
                                bias=biasT[:, ac, b : b + 1],
                            )

            def issue_logits_softmax(g):
                sco = score_tiles[g]
                ps_lg = ps_c_p.tile([97, HW], FP32, tag="psc")
                for j in range(G):
                    for ac in range(4):
                        nc.tensor.matmul(
                            ps_lg[32 * j : 32 * j + 1, :],
                            wv_bf[:, ac : ac + 1],
                            sco[:, ac, j * HW : (j + 1) * HW],
                            start=(ac == 0),
                            stop=(ac == 3),
                            tile_position=(0, 32 * j),
                        )
                st = smax_p.tile([97, 4], FP32, tag="smx")
                ex = smax_p.tile([97, HW], FP32, tag="ex")
                alpha_full = smax_p.tile([97, HW], FP32, tag="alpha")
                alpha_tiles[g] = alpha_full
                nc.vector.reduce_max(st[:, 0:1], ps_lg, axis=AX, negate=True)
                nc.scalar.activation(
                    out=ex,
                    in_=ps_lg,
                    func=AF.Exp,
                    bias=st[:, 0:1],
                    accum_out=st[:, 1:2],
                )
                nc.vector.reciprocal(st[:, 2:3], st[:, 1:2])
                nc.vector.tensor_scalar_mul(alpha_full, ex, st[:, 2:3])
                for j in range(G):
                    bg = g * G + j
                    nc.sync.dma_start(
                        out=out_d[bg : bg + 1, ENC : ENC + HW],
                        in_=alpha_full[32 * j : 32 * j + 1, :],
                    )

            def issue_alpha_t(g):
                alpha_full = alpha_tiles[g]
                # diagonal alpha at partitions {0,32,64,96}: af[32j, r] =
                # alpha_j(r - j*HW) within batch j's row range, else 0
                af = smax_p.tile([97, GR], BF16, tag="aflat")
                nc.vector.memset(af, 0.0)
                for j in range(G):
                    nc.vector.tensor_copy(
                        af[32 * j : 32 * j + 1, j * HW : (j + 1) * HW],
                        alpha_full[32 * j : 32 * j + 1, :],
                    )
                a_sb = smax_p.tile([128, len(TSIZES), G], BF16, tag="asb")
                a_tiles[g] = a_sb
                for t, p in enumerate(TSIZES):
                    ps_at = ps_c_p.tile([128, 98], FP32, tag="psc")
                    nc.tensor.matmul(
                        ps_at[0:p, 0:97],
                        af[:, t * 128 : t * 128 + p],
                        ident_bf[0:97, 0:97],
                        start=True,
                        stop=True,
                    )
                    nc.vector.tensor_copy(a_sb[0:p, t, :], ps_at[0:p, 0:97:32])

            def context_packets(g):
                a_sb = a_tiles[g]
                ctx_sb = outs_p.tile([G, ENC], FP32, tag="ctx")

                def nchunk(nch):
                    ps_c = ps_c_p.tile([G, 512], FP32, tag="psc")
                    for t, p in enumerate(TSIZES):
                        nc.tensor.matmul(
                            ps_c,
                            a_sb[0:p, t, :],
                            nat_tiles[g][t][:, nch * 512 : (nch + 1) * 512],
                            start=(t == 0),
                            stop=(t == len(TSIZES) - 1),
                        )
                    nc.scalar.copy(ctx_sb[:, nch * 512 : (nch + 1) * 512], ps_c)

                for nch in range(4):
                    yield lambda nch=nch: nchunk(nch)
                yield lambda: nc.sync.dma_start(
                    out=out_d[g * G : (g + 1) * G, 0:ENC], in_=ctx_sb
                )

            def issue_context(g):
                for pkt in context_packets(g):
                    pkt()

            issue_loads(0)
            # per-ATTN-chunk column layouts of the small vectors
            benc_t = singles.tile([128, 4], FP32)
            nc.gpsimd.dma_start(
                out=benc_t, in_=benc_d.rearrange("(ac p) -> p ac", p=128)
            )
            bdec_t = singles.tile([128, 4], FP32)
            nc.gpsimd.dma_start(
                out=bdec_t, in_=bdec_d.rearrange("(ac p) -> p ac", p=128)
            )
            bias_vec = singles.tile([128, 4], FP32)
            nc.vector.tensor_add(bias_vec, benc_t, bdec_t)

            wv_f = singles.tile([128, 4], FP32)
            nc.gpsimd.dma_start(out=wv_f, in_=wv_d.rearrange("(ac p) -> p ac", p=128))
            wv_bf = singles.tile([128, 4], BF16)
            nc.vector.tensor_copy(wv_bf, wv_f)

            # ---- attn2 / bias precompute: biasT[:, ac, b] = (dec_h @ W_dec + b_dec + b_enc)^T
            biasT = singles.tile([128, 4, BL], FP32)
            with tc.tile_pool(name="pre", bufs=1) as pre:
                dec_sb = pre.tile([BL, DEC], BF16)
                nc.gpsimd.dma_start(out=dec_sb, in_=dec_d[:, :])
                wdec_sb = pre.tile([128, 4, ATTN], BF16)
                for dc in range(4):
                    nc.gpsimd.dma_start(
                        out=wdec_sb[:, dc, :], in_=wdec_d[dc * 128 : (dc + 1) * 128, :]
                    )
                dechT = pre.tile([128, 4, BL], BF16)
                for dc in range(4):
                    ps = ps_t.tile([128, BL], FP32, tag="pst")
                    nc.tensor.matmul(
                        ps,
                        dec_sb[:, dc * 128 : (dc + 1) * 128],
                        ident_bf[0:BL, 0:BL],
                        start=True,
                        stop=True,
                    )
                    nc.vector.tensor_copy(dechT[:, dc, :], ps)
                for ac in range(4):
                    ps2 = ps_t.tile([128, BL], FP32, tag="pst")
                    for dc in range(4):
                        nc.tensor.matmul(
                            ps2,
                            wdec_sb[:, dc, ac * 128 : (ac + 1) * 128],
                            dechT[:, dc, :],
                            start=(dc == 0),
                            stop=(dc == 3),
                        )
                    nc.vector.tensor_scalar_add(
                        biasT[:, ac, :], ps2, bias_vec[:, ac : ac + 1]
                    )

            # W_enc chunks: bf16 cast-DMA load, then fp8 copy for DoubleRow matmuls
            w_bf = singles.tile([128, 16, ATTN], BF16)
            w8 = singles.tile([128, 16, ATTN], FP8)
            for kc in range(16):
                nc.gpsimd.dma_start(
                    out=w_bf[:, kc, :], in_=wenc_d[kc * 128 : (kc + 1) * 128, :]
                )
                nc.vector.tensor_copy(w8[:, kc, :], w_bf[:, kc, :])

            issue_loads(1)
            issue_transpose(0)
            for g in range(NG):
                if g + 2 < NG:
                    issue_loads(g + 2)
                issue_attn1(g)
                if g >= 1:
                    issue_alpha_t(g - 1)
                tp = transpose_packets(g + 1) if g + 1 < NG else iter(())
                cp = context_packets(g - 1) if g >= 1 else iter(())
                done = False
                while not done:
                    done = True
                    for _ in range(8):
                        pkt = next(tp, None)
                        if pkt is not None:
                            pkt()
                            done = False
                    pkt = next(cp, None)
                    if pkt is not None:
                        pkt()
                        done = False
                issue_logits_softmax(g)
            issue_alpha_t(NG - 1)
            issue_context(NG - 1)

    if not nc.is_finalized():
        nc.finalize()
    return nc


def _get_nc():
    if "nc" not in _CACHE:
        _CACHE["nc"] = build()
    return _CACHE["nc"]


def _install_ntff_hook():
    """The agent image's antenv lacks axon_hooks, so bass_utils' trace path
    can't find the NTFF profile hook. Recreate the module and install the
    ctypes-based hook from trn_agent_boot."""
    import types

    try:
        import antenv.axon_hooks  # noqa: F401
        return True
    except ImportError:
        pass
    try:
        import antenv
        from trn_agent_boot.trn_boot import _ntff_profile_via_ctypes

        hook = _ntff_profile_via_ctypes("/opt/axon/libaxon_pjrt.so")
        mod = types.ModuleType("antenv.axon_hooks")
        mod._hook = hook
        mod.set_axon_ntff_profile_hook = lambda h: setattr(mod, "_hook", h)
        mod.get_axon_ntff_profile_hook = lambda: mod._hook
        sys.modules["antenv.axon_hooks"] = mod
        antenv.axon_hooks = mod
        return hook is not None
    except Exception as e:  # pragma: no cover
        print(f"ntff hook install failed: {e}")
        return False


def run(inputs, trace=False):
    if trace:
        _install_ntff_hook()
    nc = _get_nc()
    enc = np.ascontiguousarray(inputs["enc_out"], dtype=np.float32)
    dec = np.ascontiguousarray(inputs["dec_h"], dtype=np.float32)
    shared = {
        "W_enc": np.ascontiguousarray(inputs["W_enc"], dtype=np.float32),
        "b_enc": np.ascontiguousarray(inputs["b_enc"], dtype=np.float32),
        "W_dec": np.ascontiguousarray(inputs["W_dec"], dtype=np.float32),
        "b_dec": np.ascontiguousarray(inputs["b_dec"], dtype=np.float32),
        "W_v": np.ascontiguousarray(inputs["W_v"], dtype=np.float32),
    }
    in_maps = []
    for i in range(N_CORES):
        m = dict(shared)
        m["enc_out"] = enc[i * BL : (i + 1) * BL]
        m["dec_h"] = dec[i * BL : (i + 1) * BL]
        in_maps.append(m)
    res = run_bass_kernel_spmd(nc, in_maps, core_ids=list(range(N_CORES)), trace=trace)
    outs = [res.results[i]["out"] for i in range(N_CORES)]
    full = np.concatenate(outs, axis=0)  # [512, 2244]
    context = np.ascontiguousarray(full[:, :ENC])
    alpha = np.ascontiguousarray(full[:, ENC:])
    return (context, alpha), res


def kernel(**inputs):
    (context, alpha), _ = run(inputs, trace=False)
    return (context, alpha)


# revision 40
# speedup vs baseline: 1.1079x; 1.0160x over previous
"""Trainium2 Bass kernel for Bahdanau-style attention (nn_Attention).

Computation (per batch b):
  attn1 = enc_out @ W_enc + b_enc          # [HW, ATTN]
  attn2 = dec_h @ W_dec + b_dec            # [ATTN]
  score = relu(attn1 + attn2)              # [HW, ATTN]
  logits = score @ W_v (+ b_v)             # [HW]  (b_v dropped: softmax-invariant)
  alpha = softmax(logits)                  # [HW]
  context = alpha @ enc_out                # [ENC]
Returns (context [B, ENC] f32, alpha [B, HW] f32).

Strategy: pure data-parallel over batch across 8 NeuronCores (64 batches/core),
batches processed in pipelined groups of 4:
  - enc tiles cast-DMA'd HBM f32 -> SBUF bf16 natural layout (SWDGE cast),
    as contiguous group-row tiles (6x128 + 1x16 rows per group).
  - encT (the moving operand of the big matmul) built by PE transposes written
    as REGULAR identity matmuls (exact, and unlike is_transpose they count as
    PE activity for the HAM clock gate); PSUM f32 evacuated with an fp8 cast
    split across DVE/ACT.
  - attn1^T = W_enc.T @ enc.T in fp8e4m3 with DoubleRow perf mode (2 e-chunks
    contracted per pass via the 3D-AP pair form); ~2x PE throughput vs bf16 at
    ~1.3e-2 final rel err (vs 2.5e-3 all-bf16).
  - bias (b_enc + b_dec + attn2_b) folded into the PSUM evacuation on ACT
    (per-partition bias + relu + cast to bf16 score).
  - logits via W_v-stationary matmuls; each batch's logits land on PSUM
    partition 32*j via tile_position, so softmax runs batched on one tile
    (reduce_max(negate) -> Exp with bias and fused accum_out -> reciprocal).
  - alpha transposed back to columns by one PE matmul per row-half; zero-padded
    block-diagonal A tiles kill cross-batch terms so the context accumulates a
    whole group in one PSUM group per 512-col chunk.
  - Issue order pipelines groups: loads lead by 2 groups; the LDW-heavy
    transpose packets for group g+1 are interleaved with the dense context
    matmuls of group g-1 to keep the HAM busy-fraction up; attn1 stays a dense
    fp8 block.
"""

import sys

if "/opt/trn_rl_repo" not in sys.path:
    sys.path.insert(0, "/opt/trn_rl_repo")

import numpy as np

import concourse.bass as bass
import concourse.tile as tile
from concourse import bacc, mybir
from concourse.bass_utils import run_bass_kernel_spmd
from concourse.masks import make_identity

N_CORES = 8
B, HW, ENC, DEC, ATTN = 512, 196, 2048, 512, 512
BL = B // N_CORES  # 64 batches per core
G = 4              # batches per group
NG = BL // G       # 16 groups
HW0 = 128
HW1 = HW - HW0     # 68
OUTW = ENC + HW    # context (2048) + alpha (196)

FP32 = mybir.dt.float32
BF16 = mybir.dt.bfloat16
FP8 = mybir.dt.float8e4
DR = mybir.MatmulPerfMode.DoubleRow
AX = mybir.AxisListType.X
AF = mybir.ActivationFunctionType

_CACHE = {}


def build():
    from contextlib import ExitStack

    nc = bacc.Bacc(
        "TRN2", target_bir_lowering=False, debug=False, num_devices=N_CORES
    )
    enc_d = nc.declare_dram_parameter("enc_out", [BL, HW, ENC], FP32, isOutput=False)
    dec_d = nc.declare_dram_parameter("dec_h", [BL, DEC], FP32, isOutput=False)
    wenc_d = nc.declare_dram_parameter("W_enc", [ENC, ATTN], FP32, isOutput=False)
    benc_d = nc.declare_dram_parameter("b_enc", [ATTN], FP32, isOutput=False)
    wdec_d = nc.declare_dram_parameter("W_dec", [DEC, ATTN], FP32, isOutput=False)
    bdec_d = nc.declare_dram_parameter("b_dec", [ATTN], FP32, isOutput=False)
    wv_d = nc.declare_dram_parameter("W_v", [ATTN], FP32, isOutput=False)
    out_d = nc.declare_dram_parameter("out", [BL, OUTW], FP32, isOutput=True)

    with tile.TileContext(nc) as tc:
        with ExitStack() as ctx:
            singles = ctx.enter_context(tc.tile_pool(name="singles", bufs=1))

            ident_bf = singles.tile([128, 128], BF16)
            make_identity(nc, ident_bf)

            # ---- main pools
            nat_p = ctx.enter_context(tc.tile_pool(name="nat", bufs=20))
            natS_p = ctx.enter_context(tc.tile_pool(name="natS", bufs=3))
            enct_p = ctx.enter_context(tc.tile_pool(name="enct", bufs=2))
            score_p = ctx.enter_context(tc.tile_pool(name="score", bufs=2))
            smax_p = ctx.enter_context(tc.tile_pool(name="smax", bufs=2))
            outs_p = ctx.enter_context(tc.tile_pool(name="outs", bufs=2))
            ps_a = ctx.enter_context(tc.tile_pool(name="ps_a", bufs=2, space="PSUM"))
            ps_t = ctx.enter_context(tc.tile_pool(name="ps_t", bufs=4, space="PSUM"))
            ps_c_p = ctx.enter_context(tc.tile_pool(name="ps_c", bufs=2, space="PSUM"))

            nat_tiles = {}
            enct_tiles = {}
            score_tiles = {}
            a_tiles = {}
            alpha_tiles = {}

            GR = G * HW          # 784 rows per group
            NT_FULL = GR // 128  # 6 full tiles
            TAIL = GR - NT_FULL * 128  # 16
            TSIZES = [128] * NT_FULL + [TAIL]
            encF = enc_d.rearrange("b s e -> (b s) e")

            def issue_loads(g):
                tiles = []
                for t, p in enumerate(TSIZES):
                    pool = nat_p if p == 128 else natS_p
                    nt = pool.tile([p, ENC], BF16, tag=f"nat{'S' if p < 128 else '0'}")
                    r0 = g * GR + t * 128
                    nc.gpsimd.dma_start(out=nt, in_=encF[r0 : r0 + p, :])
                    tiles.append(nt)
                nat_tiles[g] = tiles

            def transpose_packets(g):
                """Yield closures: 4 transpose matmuls + 1 evac each."""
                enct = enct_p.tile([128, 16, G * HW], FP8)
                enct_tiles[g] = enct

                def pkt(t, kq):
                    nt = nat_tiles[g][t]
                    p = TSIZES[t]
                    ps0 = ps_t.tile([128, 4 * p], FP32, tag="pst")
                    for u in range(4):
                        kc = kq * 4 + u
                        nc.tensor.matmul(
                            ps0[:, u * p : (u + 1) * p],
                            nt[:, kc * 128 : (kc + 1) * 128],
                            ident_bf[0:p, 0:p],
                            start=True,
                            stop=True,
                        )
                    ev = nc.scalar.copy if t in (1, 4) else nc.vector.tensor_copy
                    ev(
                        enct[:, kq * 4 : (kq + 1) * 4, t * 128 : t * 128 + p],
                        ps0.rearrange("p (u c) -> p u c", u=4),
                    )

                for t in range(len(TSIZES)):
                    for kq in range(4):
                        yield lambda t=t, kq=kq: pkt(t, kq)

            def issue_transpose(g):
                for pkt in transpose_packets(g):
                    pkt()

            def issue_attn1(g):
                enct = enct_tiles[g]
                sco = score_p.tile([128, 4, G * HW], BF16, tag="score")
                score_tiles[g] = sco
                half_n = G * HW // 2  # 392 = 2 batches
                for ac in range(4):
                    for half in range(2):
                        ps = ps_a.tile([128, half_n], FP32, tag="psa")
                        for kc2 in range(8):
                            nc.tensor.matmul(
                                ps,
                                w8[:, 2 * kc2 : 2 * kc2 + 2, ac * 128 : (ac + 1) * 128],
                                enct[:, 2 * kc2 : 2 * kc2 + 2, half * half_n : (half + 1) * half_n],
                                start=(kc2 == 0),
                                stop=(kc2 == 7),
                                perf_mode=DR,
                            )
                        for j2 in range(2):
                            j = half * 2 + j2
                            b = g * G + j
                            nc.scalar.activation(
                                out=sco[:, ac, j * HW : (j + 1) * HW],
                                in_=ps[:, j2 * HW : (j2 + 1) * HW],
                                func=AF.Relu,
                                bias=biasT[:, ac, b : b + 1],
                            )

            def issue_logits_softmax(g):
                sco = score_tiles[g]
                ps_lg = ps_c_p.tile([97, HW], FP32, tag="psc")
                for j in range(G):
                    for ac in range(4):
                        nc.tensor.matmul(
                            ps_lg[32 * j : 32 * j + 1, :],
                            wv_bf[:, ac : ac + 1],
                            sco[:, ac, j * HW : (j + 1) * HW],
                            start=(ac == 0),
                            stop=(ac == 3),
                            tile_position=(0, 32 * j),
                        )
                st = smax_p.tile([97, 4], FP32, tag="smx")
                ex = smax_p.tile([97, HW], FP32, tag="ex")
                alpha_full = smax_p.tile([97, HW], FP32, tag="alpha")
                alpha_tiles[g] = alpha_full
                nc.vector.reduce_max(st[:, 0:1], ps_lg, axis=AX, negate=True)
                nc.scalar.activation(
                    out=ex,
                    in_=ps_lg,
                    func=AF.Exp,
                    bias=st[:, 0:1],
                    accum_out=st[:, 1:2],
                )
                nc.vector.reciprocal(st[:, 2:3], st[:, 1:2])
                nc.vector.tensor_scalar_mul(alpha_full, ex, st[:, 2:3])
                for j in range(G):
                    bg = g * G + j
                    nc.sync.dma_start(
                        out=out_d[bg : bg + 1, ENC : ENC + HW],
                        in_=alpha_full[32 * j : 32 * j + 1, :],
                    )

            def issue_alpha_t(g):
                alpha_full = alpha_tiles[g]
                # diagonal alpha at partitions {0,32,64,96}: af[32j, r] =
                # alpha_j(r - j*HW) within batch j's row range, else 0
                af = smax_p.tile([97, GR], BF16, tag="aflat")
                nc.vector.memset(af, 0.0)
                for j in range(G):
                    nc.vector.tensor_copy(
                        af[32 * j : 32 * j + 1, j * HW : (j + 1) * HW],
                        alpha_full[32 * j : 32 * j + 1, :],
                    )
                a_sb = smax_p.tile([128, len(TSIZES), G], BF16, tag="asb")
                a_tiles[g] = a_sb
                for t, p in enumerate(TSIZES):
                    ps_at = ps_c_p.tile([128, 98], FP32, tag="psc")
                    nc.tensor.matmul(
                        ps_at[0:p, 0:97],
                        af[:, t * 128 : t * 128 + p],
                        ident_bf[0:97, 0:97],
                        start=True,
                        stop=True,
                    )
                    nc.vector.tensor_copy(a_sb[0:p, t, :], ps_at[0:p, 0:97:32])

            def context_packets(g):
                a_sb = a_tiles[g]
                ctx_sb = outs_p.tile([G, ENC], FP32, tag="ctx")

                def nchunk(nch):
                    ps_c = ps_c_p.tile([G, 512], FP32, tag="psc")
                    for t, p in enumerate(TSIZES):
                        nc.tensor.matmul(
                            ps_c,
                            a_sb[0:p, t, :],
                            nat_tiles[g][t][:, nch * 512 : (nch + 1) * 512],
                            start=(t == 0),
                            stop=(t == len(TSIZES) - 1),
                        )
                    nc.scalar.copy(ctx_sb[:, nch * 512 : (nch + 1) * 512], ps_c)

                for nch in range(4):
                    yield lambda nch=nch: nchunk(nch)
                yield lambda: nc.sync.dma_start(
                    out=out_d[g * G : (g + 1) * G, 0:ENC], in_=ctx_sb
                )

            def issue_context(g):
                for pkt in context_packets(g):
                    pkt()

            issue_loads(0)
            # per-ATTN-chunk column layouts of the small vectors
            benc_t = singles.tile([128, 4], FP32)
            nc.gpsimd.dma_start(
                out=benc_t, in_=benc_d.rearrange("(ac p) -> p ac", p=128)
            )
            bdec_t = singles.tile([128, 4], FP32)
            nc.gpsimd.dma_start(
                out=bdec_t, in_=bdec_d.rearrange("(ac p) -> p ac", p=128)
            )
            bias_vec = singles.tile([128, 4], FP32)
            nc.vector.tensor_add(bias_vec, benc_t, bdec_t)

            wv_f = singles.tile([128, 4], FP32)
            nc.gpsimd.dma_start(out=wv_f, in_=wv_d.rearrange("(ac p) -> p ac", p=128))
            wv_bf = singles.tile([128, 4], BF16)
            nc.vector.tensor_copy(wv_bf, wv_f)

            # ---- attn2 / bias precompute: biasT[:, ac, b] = (dec_h @ W_dec + b_dec + b_enc)^T
            biasT = singles.tile([128, 4, BL], FP32)
            with tc.tile_pool(name="pre", bufs=1) as pre:
                dec_sb = pre.tile([BL, DEC], BF16)
                nc.gpsimd.dma_start(out=dec_sb, in_=dec_d[:, :])
                wdec_sb = pre.tile([128, 4, ATTN], BF16)
                for dc in range(4):
                    nc.gpsimd.dma_start(
                        out=wdec_sb[:, dc, :], in_=wdec_d[dc * 128 : (dc + 1) * 128, :]
                    )
                dechT = pre.tile([128, 4, BL], BF16)
                for dc in range(4):
                    ps = ps_t.tile([128, BL], FP32, tag="pst")
                    nc.tensor.matmul(
                        ps,
                        dec_sb[:, dc * 128 : (dc + 1) * 128],
                        ident_bf[0:BL, 0:BL],
                        start=True,
                        stop=True,
                    )
                    nc.vector.tensor_copy(dechT[:, dc, :], ps)
                for ac in range(4):
                    ps2 = ps_t.tile([128, BL], FP32, tag="pst")
                    for dc in range(4):
                        nc.tensor.matmul(
                            ps2,
                            wdec_sb[:, dc, ac * 128 : (ac + 1) * 128],
                            dechT[:, dc, :],
                            start=(dc == 0),
                            stop=(dc == 3),
                        )
                    nc.vector.tensor_scalar_add(
                        biasT[:, ac, :], ps2, bias_vec[:, ac : ac + 1]
                    )

            # W_enc chunks: bf16 cast-DMA load, then fp8 copy for DoubleRow matmuls
            w_bf = singles.tile([128, 16, ATTN], BF16)
            w8 = singles.tile([128, 16, ATTN], FP8)
            for kc in range(16):
                nc.gpsimd.dma_start(
                    out=w_bf[:, kc, :], in_=wenc_d[kc * 128 : (kc + 1) * 128, :]
                )
                nc.vector.tensor_copy(w8[:, kc, :], w_bf[:, kc, :])

            issue_loads(1)
            issue_transpose(0)
            for g in range(NG):
                if g + 2 < NG:
                    issue_loads(g + 2)
                issue_attn1(g)
                if g >= 1:
                    issue_alpha_t(g - 1)
                tp = transpose_packets(g + 1) if g + 1 < NG else iter(())
                cp = context_packets(g - 1) if g >= 1 else iter(())
                done = False
                while not done:
                    done = True
                    for _ in range(8):
                        pkt = next(tp, None)
                        if pkt is not None:
                            pkt()
                            done = False
                    pkt = next(cp, None)
                    if pkt is not None:
                        pkt()
                        done = False
                issue_logits_softmax(g)
            issue_alpha_t(NG - 1)
            issue_context(NG - 1)

    if not nc.is_finalized():
        nc.finalize()
    return nc


def _get_nc():
    if "nc" not in _CACHE:
        _CACHE["nc"] = build()
    return _CACHE["nc"]


def _install_ntff_hook():
    """The agent image's antenv lacks axon_hooks, so bass_utils' trace path
    can't find the NTFF profile hook. Recreate the module and install the
    ctypes-based hook from trn_agent_boot."""
    import types

    try:
        import antenv.axon_hooks  # noqa: F401
        return True
    except ImportError:
        pass
    try:
        import antenv
        from trn_agent_boot.trn_boot import _ntff_profile_via_ctypes

        hook = _ntff_profile_via_ctypes("/opt/axon/libaxon_pjrt.so")
        mod = types.ModuleType("antenv.axon_hooks")
        mod._hook = hook
        mod.set_axon_ntff_profile_hook = lambda h: setattr(mod, "_hook", h)
        mod.get_axon_ntff_profile_hook = lambda: mod._hook
        sys.modules["antenv.axon_hooks"] = mod
        antenv.axon_hooks = mod
        return hook is not None
    except Exception as e:  # pragma: no cover
        print(f"ntff hook install failed: {e}")
        return False


def run(inputs, trace=False):
    if trace:
        _install_ntff_hook()
    nc = _get_nc()
    enc = np.ascontiguousarray(inputs["enc_out"], dtype=np.float32)
    dec = np.ascontiguousarray(inputs["dec_h"], dtype=np.float32)
    shared = {
        "W_enc": np.ascontiguousarray(inputs["W_enc"], dtype=np.float32),
        "b_enc": np.ascontiguousarray(inputs["b_enc"], dtype=np.float32),
        "W_dec": np.ascontiguousarray(inputs["W_dec"], dtype=np.float32),
        "b_dec": np.ascontiguousarray(inputs["b_dec"], dtype=np.float32),
        "W_v": np.ascontiguousarray(inputs["W_v"], dtype=np.float32),
    }
    in_maps = []
    for i in range(N_CORES):
        m = dict(shared)
        m["enc_out"] = enc[i * BL : (i + 1) * BL]
        m["dec_h"] = dec[i * BL : (i + 1) * BL]
        in_maps.append(m)
    res = run_bass_kernel_spmd(nc, in_maps, core_ids=list(range(N_CORES)), trace=trace)
    outs = [res.results[i]["out"] for i in range(N_CORES)]
    full = np.concatenate(outs, axis=0)  # [512, 2244]
    context = np.ascontiguousarray(full[:, :ENC])
    alpha = np.ascontiguousarray(full[:, ENC:])
    return (context, alpha), res


def kernel(**inputs):
    (context, alpha), _ = run(inputs, trace=False)
    return (context, alpha)
